# revision 8
# baseline (speedup 1.0000x reference)
"""DGCNN point-cloud classifier forward pass on 8 Trainium2 NeuronCores.

Data-parallel over batch: each core processes one point cloud (B=8, N=1024).
All feature maps are kept channel-major (C x N) in SBUF. Edge-conv layers:
  knn scores via PE matmul, top-20 via DVE max8/max_index/match_replace,
  neighbor feature max via GPSIMD ap_gather (SBUF column gather) + contiguous
  DVE reduce_max. BN scale is folded into the conv weights on the host
  (valid because all BN gammas are positive, so max commutes with BN+ELU).

The per-tile index table for ap_gather is built with two DMAs (one flat
write of the top-20 indices to DRAM, one wrapped broadcast read back into
the [128, 160] core-replicated layout), and reduce emission is deferred a
tile behind the gathers so the in-order DVE/Pool streams never wait on the
DMA roundtrip.
"""
import sys

for _p in ("/opt/trn_rl_repo", "/root/.axon_site/_ro/trn_rl_repo"):
    if _p not in sys.path:
        sys.path.insert(0, _p)

import numpy as np

import concourse.bacc as bacc
import concourse.mybir as mybir
import concourse.tile as tile
from concourse import library_config
from concourse.bass_types import AP
from concourse.bass_utils import run_bass_kernel_spmd

F32 = mybir.dt.float32
F32R = mybir.dt.float32r
U16 = mybir.dt.uint16
I16 = mybir.dt.int16
AF = mybir.ActivationFunctionType
AX = mybir.AxisListType

N = 1024
K = 20
NT = 8          # point tiles of 128
P = 128
NEG = -1e30
EPS = 1e-5
BN_SCALE = float(1.0 / np.sqrt(1.0 + EPS))

# (C_in, O_out, split_points_across_partition_halves)
LAYERS = [(3, 64, True), (64, 64, True), (64, 128, False), (128, 256, False)]

_CACHE = {}


def _build(debug=False):
    nc = bacc.Bacc("TRN2", target_bir_lowering=False, debug=False)

    ins = {}

    def dram_in(name, shape, dt=F32):
        ins[name] = nc.dram_tensor(name, list(shape), dt, kind="ExternalInput")
        return ins[name]

    xT_in = dram_in("xT", (3, N))
    for li, (C, O, _s) in enumerate(LAYERS):
        dram_in(f"wl{li}", (C, O))
        dram_in(f"wv{li}", (C, O))
        dram_in(f"bb{li}", (1, O))
    dram_in("W5s", (P, 4, 1024), F32R)
    dram_in("b5", (1, 1024), F32R)
    dram_in("Wl1s", (P, 16, 512), F32R)
    dram_in("bl1", (1, 512), F32R)
    dram_in("Wl2s", (P, 4, 256), F32R)
    dram_in("bl2", (1, 256), F32R)
    dram_in("Wl3s", (P, 2, 40), F32R)
    dram_in("bl3", (1, 40), F32R)

    out_t = nc.dram_tensor("out", [1, 40], F32, kind="ExternalOutput")
    dbg = {}
    if debug:
        for li, (C, O, _s) in enumerate(LAYERS):
            dbg[f"xo{li}"] = nc.dram_tensor(f"xo{li}", [O, N], F32, kind="ExternalOutput")
            dbg[f"idx{li}"] = nc.dram_tensor(f"idx{li}", [P, NT, 24], U16, kind="ExternalOutput")
        dbg["f5"] = nc.dram_tensor("f5", [P, 16], F32, kind="ExternalOutput")

    with tile.TileContext(nc) as tc:
        with tc.tile_pool(name="persist", bufs=1) as pp, \
             tc.tile_pool(name="work", bufs=1) as wp, \
             tc.tile_pool(name="sco", bufs=1) as sco, \
             tc.tile_pool(name="gatp", bufs=2) as gatp, \
             tc.tile_pool(name="wstr", bufs=2) as wstr, \
             tc.tile_pool(name="dram", bufs=1, space="DRAM") as dp, \
             tc.tile_pool(name="ps1", bufs=1, space="PSUM") as ps1, \
             tc.tile_pool(name="pss", bufs=1, space="PSUM") as pss:

            def mmr(out, lhsT, rhs, **kw):
                nc.tensor.matmul(out, lhsT=lhsT.bitcast(F32R),
                                 rhs=rhs.bitcast(F32R), **kw)

            # ---------------- constants & weights ----------------
            ones_f = pp.tile([1, N], F32)
            nc.vector.memset(ones_f, 1.0)
            ones_row = pp.tile([1, N], F32R)
            nc.scalar.copy(ones_row, ones_f)
            neghalf = pp.tile([P, 1], F32)
            nc.vector.memset(neghalf, -0.5)

            xT0 = pp.tile([3, N], F32)
            nc.sync.dma_start(out=xT0, in_=xT_in[:, :])

            wl = {}
            wv = {}
            bb = {}

            def load_conv_weights(li):
                C, O, _s = LAYERS[li]
                wl[li] = pp.tile([C, O], F32, tag=f"wl{li}", name=f"wl{li}")
                nc.sync.dma_start(out=wl[li], in_=ins[f"wl{li}"][:, :])
                wv[li] = pp.tile([C, O], F32, tag=f"wv{li}", name=f"wv{li}")
                nc.sync.dma_start(out=wv[li], in_=ins[f"wv{li}"][:, :])
                bb[li] = pp.tile([1, O], F32, tag=f"bb{li}", name=f"bb{li}")
                nc.sync.dma_start(out=bb[li], in_=ins[f"bb{li}"][:, :])

            # feature tiles (channel-major)
            x1T = pp.tile([64, N], F32)
            x2T = pp.tile([64, N], F32)
            x12T = pp.tile([P, N], F32R)       # [x1; x2] assembled for W5
            x3T = pp.tile([P, N], F32)
            x4T = pp.tile([P, 2, N], F32)

            nc.gpsimd.load_library(library_config.ap_gather)
            load_conv_weights(0)

            # ---------------- edge conv layers ----------------
            def edge_conv(li, xT, C, O, split, outs):
                """xT: AP [C, N] input features (channel-major).
                outs: list of APs ([om, N]) to write the layer output tiles."""
                n_ot = (O + P - 1) // P
                om = min(O, P)

                # xxn = -0.5 * ||x_m||^2  (row [1, N])
                sq = wp.tile([C, N], F32, tag="sq")
                nc.scalar.activation(sq, xT, AF.Square)
                xxn_ps = ps1.tile([1, N], F32, space="PSUM", tag="misc_ps", name="xxn_ps")
                for h in range(2):
                    hs = slice(h * 512, (h + 1) * 512)
                    nc.tensor.matmul(xxn_ps[:, hs], lhsT=neghalf[0:C, :], rhs=sq[:, hs], start=True, stop=True)
                xxn = wp.tile([1, N], F32, tag="xxn")
                nc.scalar.copy(xxn, xxn_ps)

                # u^T = Wl' x  (per o-tile), duplicated across halves if split
                uts = []
                for ot in range(n_ot):
                    osl = slice(ot * P, ot * P + om)
                    u_ps = ps1.tile([om, N], F32, space="PSUM", tag="uv_ps")
                    for h in range(2):
                        hs = slice(h * 512, (h + 1) * 512)
                        nc.tensor.matmul(u_ps[:, hs], lhsT=wl[li][:, osl],
                                         rhs=xT[:, hs], start=True, stop=True)
                    ut = wp.tile([P, N], F32, tag=f"ut{ot}")
                    nc.scalar.copy(ut[0:om, :], u_ps)
                    if split:
                        nc.scalar.copy(ut[64:128, :], ut[0:64, :])
                    uts.append(ut)

                # top-k per point tile -> one flat idx write + one wrapped
                # broadcast read builds the core-replicated gather table.
                vals = wp.tile([P, 8], F32, tag="vals")
                QC = 2560          # gathered columns per ap_gather
                m1s = [wp.tile([P, (N // (2 if split else 1))], F32,
                               tag=f"m1_{ot}", name=f"m1_{ot}")
                       for ot in range(n_ot)]
                order = [0, 4, 1, 5, 2, 6, 3, 7] if split else list(range(NT))
                didx = {}
                rq = []

                def do_topk(t):
                    tsl = slice(t * P, (t + 1) * P)
                    s_ps = pss.tile([P, N], F32, space="PSUM", tag="s_ps",
                                    name="s_ps", bufs=2)
                    for h in range(2):
                        hs = slice(h * 512, (h + 1) * 512)
                        nc.tensor.matmul(s_ps[:, hs], lhsT=xT[:, tsl],
                                         rhs=xT[:, hs], start=True, stop=False)
                        nc.tensor.matmul(s_ps[:, hs], lhsT=ones_f[:, 0:P],
                                         rhs=xxn[:, hs], start=False, stop=True)
                    s_sb = sco.tile([P, N], F32, tag="s_sb", name="s_sb", bufs=3)
                    nc.scalar.copy(s_sb, s_ps)
                    ii = wp.tile([P, 24], U16, tag="idx_t", name="idx_t", bufs=6)
                    for r in range(3):
                        nc.vector.max(out=vals, in_=s_sb)
                        nc.vector.max_index(out=ii[:, r * 8:(r + 1) * 8],
                                            in_max=vals, in_values=s_sb)
                        if r < 2:
                            nc.vector.match_replace(out=s_sb, in_to_replace=vals,
                                                    in_values=s_sb, imm_value=NEG)
                    if debug:
                        nc.sync.dma_start(out=dbg[f"idx{li}"][:, t, :], in_=ii)
                    # top-20 indices to DRAM, pre-wrapped for the gather table:
                    # dram[r, 20*(m//16) + j] = idx[m, j] for r = m % 16 (the
                    # per-core index stream is q = 320*(m//16) + 16*j + m%16).
                    # 8 block writes (one clean 2D descriptor each, spread over
                    # the DMA queues); a single 3-dim DMA lowers to 128 one-run
                    # descriptors on one queue (~64us serial) - measured.
                    dr = dp.tile([16, 160], U16, tag="didx", name="didx", bufs=8)
                    didx[t] = dr
                    da = dr[:]
                    for a in range(8):
                        wr = AP(tensor=da.tensor, offset=da.offset + a * 20,
                                ap=[da.ap[0], [1, 20]])
                        eng = nc.sync if a % 2 == 0 else nc.scalar
                        eng.dma_start(out=wr, in_=ii[16 * a:16 * (a + 1), 0:20])

                def do_gather(ts):
                    # broadcast the wrapped [16, 160] table to all 8 cores'
                    # partition blocks (contiguous read, stride-0 partition dim)
                    tbl = wp.tile([P, 160], U16, tag="tbl", name="tbl", bufs=4)
                    if split:
                        tlo, thi = ts
                        for h, tt in ((0, tlo), (1, thi)):
                            dr = didx[tt][:]
                            rd = AP(tensor=dr.tensor, offset=dr.offset,
                                    ap=[[0, 4], [160, 16], [1, 160]])
                            nc.sync.dma_start(out=tbl[64 * h:64 * (h + 1), :], in_=rd)
                        mcol = (tlo % 4) * P
                    else:
                        dr = didx[ts][:]
                        rd = AP(tensor=dr.tensor, offset=dr.offset,
                                ap=[[0, 8], [160, 16], [1, 160]])
                        nc.sync.dma_start(out=tbl, in_=rd)
                        mcol = ts * P
                    for ot in range(n_ot):
                        gat = gatp.tile([P, QC], F32, tag="gat", name="gat", bufs=2)
                        nc.gpsimd.ap_gather(
                            gat.rearrange("p (q d) -> p q d", d=1),
                            uts[ot].rearrange("p (n d) -> p n d", d=1),
                            tbl[:, :].bitcast(I16),
                            channels=P, num_elems=N, d=1, num_idxs=QC)
                        g = gat[:]
                        view = AP(tensor=g.tensor, offset=g.offset,
                                  ap=[g.ap[0], [320, 8], [1, 16], [16, 20]])
                        rq.append((m1s[ot][:, mcol:mcol + P], view))

                def flush_rq(keep):
                    while len(rq) > keep:
                        dst, view = rq.pop(0)
                        nc.vector.reduce_max(dst, view, axis=AX.X)

                # software pipeline: gather one group behind topk, reduce one
                # gather-group behind that (keeps the DVE stream stall-free).
                pend = []
                for i_, t in enumerate(order):
                    do_topk(t)
                    if split:
                        if i_ % 2 == 1:
                            pend.append((order[i_ - 1], t))
                    else:
                        pend.append(t)
                    if len(pend) >= 2:
                        do_gather(pend.pop(0))
                    flush_rq(n_ot)
                while pend:
                    do_gather(pend.pop(0))
                flush_rq(0)

                # v^T + bias, then z = m1 + v, y = relu(z) + exp(min(z,0)) - 1
                for ot in range(n_ot):
                    osl = slice(ot * P, ot * P + om)
                    v_ps = ps1.tile([om, N], F32, space="PSUM", tag="uv_ps")
                    for h in range(2):
                        hs = slice(h * 512, (h + 1) * 512)
                        nc.tensor.matmul(v_ps[:, hs], lhsT=wv[li][:, osl],
                                         rhs=xT[:, hs], start=True, stop=False)
                        nc.tensor.matmul(v_ps[:, hs], lhsT=bb[li][:, osl],
                                         rhs=ones_f[:, 0:512], start=False, stop=True)
                    if split:
                        m1u = wp.tile([64, N], F32, tag="m1u")
                        nc.scalar.copy(m1u[:, 0:512], m1s[ot][0:64, :])
                        nc.scalar.copy(m1u[:, 512:1024], m1s[ot][64:128, :])
                        msrc = m1u
                    else:
                        msrc = m1s[ot]
                    z = wp.tile([om, N], F32, tag="z")
                    nc.vector.tensor_add(z, msrc[0:om, :], v_ps)
                    rn = wp.tile([om, N], F32, tag="rn")
                    nc.scalar.activation(rn, z, AF.Relu, scale=-1.0)
                    ee = wp.tile([om, N], F32, tag="ee")
                    nc.scalar.activation(ee, rn, AF.Exp, scale=-1.0)
                    nc.vector.scalar_tensor_tensor(
                        out=z, in0=z, scalar=-1.0, in1=rn,
                        op0=mybir.AluOpType.add, op1=mybir.AluOpType.add)
                    nc.vector.tensor_add(outs[ot], z, ee)

            load_conv_weights(1)
            edge_conv(0, xT0[:], 3, 64, True, [x1T[:, :]])
            if debug:
                nc.sync.dma_start(out=dbg["xo0"][:, :], in_=x1T[:, :].bitcast(F32))
            # FC-head weights: issue on the Activation DMA queue so the big
            # transfers never queue ahead of the per-tile index-table DMAs.
            W5s = pp.tile([P, 4, 1024], F32R)
            nc.scalar.dma_start(out=W5s, in_=ins["W5s"][:, :, :])
            b5 = pp.tile([1, 1024], F32R)
            nc.scalar.dma_start(out=b5, in_=ins["b5"][:, :])
            Wl2s = pp.tile([P, 4, 256], F32R)
            nc.scalar.dma_start(out=Wl2s, in_=ins["Wl2s"][:, :, :])
            bl2 = pp.tile([1, 256], F32R)
            nc.scalar.dma_start(out=bl2, in_=ins["bl2"][:, :])
            Wl3s = pp.tile([P, 2, 40], F32R)
            nc.scalar.dma_start(out=Wl3s, in_=ins["Wl3s"][:, :, :])
            bl3 = pp.tile([1, 40], F32R)
            nc.scalar.dma_start(out=bl3, in_=ins["bl3"][:, :])
            load_conv_weights(2)
            edge_conv(1, x1T[:, :], 64, 64, True, [x2T[:, :]])
            if debug:
                nc.sync.dma_start(out=dbg["xo1"][:, :], in_=x2T[:, :].bitcast(F32))
            bl1 = pp.tile([1, 512], F32R)
            nc.scalar.dma_start(out=bl1, in_=ins["bl1"][:, :])
            w1c = []
            for c in range(16):
                wt = wstr.tile([P, 512], F32R, tag="w1c", name=f"w1c{c}", bufs=16)
                nc.scalar.dma_start(out=wt, in_=ins["Wl1s"][:, c, :])
                w1c.append(wt[:, :])
            load_conv_weights(3)
            edge_conv(2, x2T[:, :], 64, 128, False, [x3T[:, :]])
            if debug:
                nc.sync.dma_start(out=dbg["xo2"][:, :], in_=x3T[:, :].bitcast(F32))
            edge_conv(3, x3T[:, :], 128, 256, False,
                      [x4T[:, 0, :], x4T[:, 1, :]])
            if debug:
                nc.sync.dma_start(out=dbg["xo3"][:, :],
                                  in_=x4T.rearrange("p a b -> p (a b)").bitcast(F32))

            # ---------------- W5 stage + global pooling ----------------
            nc.scalar.copy(x12T[0:64, :], x1T[:, :])
            nc.scalar.copy(x12T[64:128, :], x2T[:, :])
            x3r = pp.tile([P, N], F32R)
            nc.scalar.copy(x3r, x3T[:, :])
            x4r = pp.tile([P, 2, N], F32R)
            nc.scalar.copy(x4r[:, 0, :], x4T[:, 0, :])
            nc.scalar.copy(x4r[:, 1, :], x4T[:, 1, :])
            cat_chunks = [x12T[:, :], x3r[:, :], x4r[:, 0, :], x4r[:, 1, :]]
            hmax8 = pp.tile([P, 8], F32)
            hsum8 = pp.tile([P, 8], F32)
            srn8 = pp.tile([P, 8], F32)
            se8 = pp.tile([P, 8], F32)
            for ot in range(8):
                osl = slice(ot * P, (ot + 1) * P)
                h_ps = pss.tile([P, N], F32, space="PSUM", tag="s_ps",
                                name="h_ps", bufs=2)
                for h in range(2):
                    hs = slice(h * 512, (h + 1) * 512)
                    for c in range(4):
                        mmr(h_ps[:, hs], lhsT=W5s[:, c, osl],
                                         rhs=cat_chunks[c][:, hs],
                                         start=(c == 0), stop=False)
                    mmr(h_ps[:, hs], lhsT=b5[:, osl],
                                     rhs=ones_row[:, 0:512], start=False, stop=True)
                nc.vector.reduce_max(hmax8[:, ot:ot + 1], h_ps, axis=AX.X)
                nc.vector.reduce_sum(hsum8[:, ot:ot + 1], h_ps, axis=AX.X)
                rn5 = wp.tile([P, N], F32, tag="rn5")
                nc.scalar.activation(rn5, h_ps, AF.Relu, scale=-1.0,
                                     accum_out=srn8[:, ot:ot + 1])
                e5 = wp.tile([P, N], F32, tag="e5")
                nc.scalar.activation(e5, rn5, AF.Exp, scale=-1.0,
                                     accum_out=se8[:, ot:ot + 1])

            # x5 = ELU(hmax8); x6_raw = hsum8 + srn8 + se8 - N  (scaled by 1/N
            # folded into Wl1s host-side)
            rnm = pp.tile([P, 8], F32)
            nc.scalar.activation(rnm, hmax8, AF.Relu, scale=-1.0)
            emm = pp.tile([P, 8], F32)
            nc.scalar.activation(emm, rnm, AF.Exp, scale=-1.0)
            x5f = pp.tile([P, 8], F32R)
            nc.vector.scalar_tensor_tensor(
                out=x5f, in0=hmax8, scalar=-1.0, in1=rnm,
                op0=mybir.AluOpType.add, op1=mybir.AluOpType.add)
            nc.vector.tensor_add(x5f, x5f, emm)
            x6f = pp.tile([P, 8], F32R)
            nc.vector.tensor_add(x6f, hsum8, srn8)
            nc.vector.scalar_tensor_tensor(
                out=x6f, in0=x6f, scalar=float(-N), in1=se8,
                op0=mybir.AluOpType.add, op1=mybir.AluOpType.add)
            if debug:
                f5dbg = pp.tile([P, 16], F32)
                nc.scalar.copy(f5dbg[:, 0:8], x5f)
                nc.scalar.copy(f5dbg[:, 8:16], x6f)
                nc.sync.dma_start(out=dbg["f5"][:, :], in_=f5dbg)

            # ---------------- FC head ----------------
            def fc(in_cols, wts, bias_row, width):
                """in_cols: list of [128,1] APs (K chunks). Returns psum [1, width]."""
                f_ps = ps1.tile([1, width], F32, space="PSUM", tag="misc_ps", name="fc_ps")
                nb = (width + 511) // 512
                for b_ in range(nb):
                    ws = slice(b_ * 512, min(width, (b_ + 1) * 512))
                    for ci, col in enumerate(in_cols):
                        mmr(f_ps[:, ws], lhsT=col,
                                         rhs=wts[ci][:, ws],
                                         start=(ci == 0), stop=False)
                    mmr(f_ps[:, ws], lhsT=ones_row[:, 0:1],
                                     rhs=bias_row[:, ws], start=False, stop=True)
                return f_ps

            def elu_row(z_ps, width, tag):
                zz = pp.tile([1, width], F32R, tag=tag + "z")
                rr = pp.tile([1, width], F32, tag=tag + "r")
                ex = pp.tile([1, width], F32, tag=tag + "e")
                nc.scalar.activation(rr, z_ps, AF.Relu, scale=-1.0)
                nc.scalar.activation(ex, rr, AF.Exp, scale=-1.0)
                nc.vector.scalar_tensor_tensor(
                    out=zz, in0=z_ps, scalar=-1.0, in1=rr,
                    op0=mybir.AluOpType.add, op1=mybir.AluOpType.add)
                nc.vector.tensor_add(zz, zz, ex)
                return zz

            def to_cols(row, width, tag):
                cols = []
                for c in range(width // P):
                    cp = ps1.tile([P, 1], F32, space="PSUM", tag="misc_ps", name=tag + "p")
                    nc.tensor.matmul(cp, lhsT=row[:, c * P:(c + 1) * P].bitcast(F32),
                                     rhs=ones_f[:, 0:1],
                                     start=True, stop=True)
                    cs = pp.tile([P, 1], F32R, tag=f"{tag}c{c}", name=f"{tag}c{c}")
                    nc.scalar.copy(cs, cp)
                    cols.append(cs[:, :])
                return cols

            f_cols = [x5f[:, c:c + 1] for c in range(8)] + \
                     [x6f[:, c:c + 1] for c in range(8)]
            f1_ps = fc(f_cols, w1c, bl1[:], 512)
            f1 = elu_row(f1_ps, 512, "f1")
            c1 = to_cols(f1, 512, "c1")
            w2c = [Wl2s[:, c, :] for c in range(4)]
            f2_ps = fc(c1, w2c, bl2[:], 256)
            f2 = elu_row(f2_ps, 256, "f2")
            c2 = to_cols(f2, 256, "c2")
            w3c = [Wl3s[:, c, :] for c in range(2)]
            f3_ps = fc(c2, w3c, bl3[:], 40)
            f3 = pp.tile([1, 40], F32)
            nc.scalar.copy(f3, f3_ps)
            nc.sync.dma_start(out=out_t[:, :], in_=f3)

    nc.compile()
    return nc


def get_nc(debug=False):
    key = ("dbg" if debug else "std")
    if key not in _CACHE:
        _CACHE[key] = _build(debug)
    return _CACHE[key]


def _prep_maps(inputs, n_cores=8):
    ii = {k: np.asarray(v) for k, v in inputs.items()}
    assert int(ii["k"]) == K
    x = ii["x"].astype(np.float32)          # (8, 1024, 3)
    B = x.shape[0]
    assert B == n_cores and x.shape[1] == N

    common = {}
    convs = [("W1", "g1", "b1"), ("W2", "g2", "b2"),
             ("W3", "g3", "b3"), ("W4", "g4", "b4")]
    for li, ((C, O, _s), (wn, gn, bn)) in enumerate(zip(LAYERS, convs)):
        W = ii[wn].astype(np.float64)       # (O, 2C)
        g = ii[gn].astype(np.float64)
        b = ii[bn].astype(np.float64)
        a = g * BN_SCALE
        assert (a > 0).all(), "BN scale must be positive for max/ELU commute"
        Wlp = (a[:, None] * W[:, :C]).T      # (C, O)
        Wvp = (a[:, None] * (W[:, C:] - W[:, :C])).T
        common[f"wl{li}"] = Wlp.astype(np.float32)
        common[f"wv{li}"] = Wvp.astype(np.float32)
        common[f"bb{li}"] = b.astype(np.float32)[None, :]

    a5 = ii["g5"].astype(np.float64) * BN_SCALE
    W5 = (a5[:, None] * ii["W5"].astype(np.float64)).astype(np.float32)  # (1024, 512)
    common["W5s"] = W5.T.reshape(4, 128, 1024).transpose(1, 0, 2).copy()
    common["b5"] = ii["b5"].astype(np.float32)[None, :]

    a_l1 = ii["gl1"].astype(np.float64) * BN_SCALE
    Wl1 = (a_l1[:, None] * ii["Wl1"].astype(np.float64))                # (512, 2048)
    Wl1[:, 1024:] /= float(N)   # x6 = raw/N folding
    common["Wl1s"] = Wl1.astype(np.float32).T.reshape(16, 128, 512).transpose(1, 0, 2).copy()
    common["bl1"] = ii["bl1"].astype(np.float32)[None, :]

    a_l2 = ii["gl2"].astype(np.float64) * BN_SCALE
    Wl2 = (a_l2[:, None] * ii["Wl2"].astype(np.float64)).astype(np.float32)  # (256, 512)
    common["Wl2s"] = Wl2.T.reshape(4, 128, 256).transpose(1, 0, 2).copy()
    common["bl2"] = ii["bl2"].astype(np.float32)[None, :]

    Wl3 = ii["Wl3"].astype(np.float32)                                  # (40, 256)
    common["Wl3s"] = Wl3.T.reshape(2, 128, 40).transpose(1, 0, 2).copy()
    common["bl3"] = ii["bl3"].astype(np.float32)[None, :]

    common = {k: np.ascontiguousarray(v) for k, v in common.items()}
    in_maps = []
    for i in range(B):
        m = dict(common)
        m["xT"] = np.ascontiguousarray(x[i].T)    # (3, 1024)
        in_maps.append(m)
    return in_maps


def run(inputs, debug=False, trace=False):
    nc = get_nc(debug)
    in_maps = _prep_maps(inputs)
    res = run_bass_kernel_spmd(nc, in_maps, core_ids=list(range(8)), trace=trace)
    out = np.stack([res.results[i]["out"][0] for i in range(8)]).astype(np.float32)
    return out, res


def kernel(**inputs):
    out, _ = run(inputs)
    return out
